# revision 1
# baseline (speedup 1.0000x reference)
"""CDD loss kernel for 8 Trainium2 NeuronCores (Bass/Tile, SPMD).

Math (validated vs reference in float32):
  ps is one-hot -> every (C,C,N,N) reference tensor collapses to per-class-
  block sums. Host sorts+pads src rows by class (CAP rows/class, pads are
  huge distinct sentinel vectors so exp(-dist/bw) underflows to exactly 0).
  The E_pp class-diagonal blocks have their diagonal zeroed on device, making
  each diagonal entry contribute exactly exp(0)=1 per bandwidth; the exact
  correction (5*CAP - 5*exp(-1e-5)*cs) is applied as a host-computed offset.
  g2 is symmetric -> T2 = T1^T, so inter = sum_{s!=t} 2*(T1-T3)/(C^2-C).

Distribution (SPMD, one program, per-core data):
  - NO collective: an 8-core AllReduce has a ~95us floor in this
    environment, far more than recomputing the [12,12] global sums
    locally. Every core computes the FULL (rotated) E_ss (6 slabs),
    E_tt, and E_pt (6 slabs) in bf16 and derives the gamma sums
    S1 = W^T E_ss W, stt = pt^T E_tt pt, sst = W^T E_st pt itself
    via transpose-free two-stage small matmuls.
  - host precomputes the feature transposes (bf16) and row norms (f32);
    the device does Gram matmuls (bf16 in, f32 accum), one f32 STT for
    d2-partial, and sqrt(x + rownorm_eps) via ACT bias -> E in bf16.
    A +0.5 epsilon in the per-partition norms keeps sqrt's argument
    positive under bf16 rounding (E error ~0.25/E, negligible).
  - exp work stays sharded: each core exponentiates only its own class
    pair's T1/T3/k1/k3 blocks (flat broadcast gathers through DRAM in
    class-block layout, contiguous segments, SP+Act HWDGE queues) and
    its 5+3 (class,bandwidth) k2 units (balanced via a host-permuted
    bandwidth table folded into the ibg build).
  - per-core weighted reduce with host weight matrix -> [intra, inter]
    partials, host sums the 8 partials.
"""

import math
import numpy as np

C = 12
KN = 5
MU = 2
N = 384
D = 256
CAP = 64
R = C * CAP            # 768 padded src rows
NCORES = 8
NCOL = 20              # ACC columns: T1, T3, k1*5, k3*5, k2q0*5, k2q1*3
DIAG5 = 5.0 * math.exp(-1e-5)
I2 = 2.0 / (C * C - C)
EPS = 0.5              # d2 positivity epsilon folded into the col norms

# f32 misc pack column offsets ([128, MISCW])
O_WM = 0
O_ONES = 20
O_ONESR = 21          # row 0: ones [1,128]
O_RSCOL6 = 149        # [128,6] per-slab src col norms (+EPS)
O_RTCOL = 155         # [128,3] tgt col norms (+EPS)
O_RSROW = 158         # row 0: [1,768]
O_RTROW = 926         # row 0: [1,384]
O_EYE12 = 1310
O_OH2 = 1322
O_K2SEL = 1324
O_PERM65 = 1326       # rows 0-64
O_PW60 = 1391
O_PW10 = 1451
O_RDEN2 = 1461
O_RDENIN = 1473
O_SSEL = 1474         # rows 0-19
O_OFFS = 1476         # row 0
MISCW = 1478

# bf16 miscb pack column offsets ([128, MISCBW])
B_WR = 0              # 6 x [128,12]
B_PTB = 72            # 3 x [128,12]
B_DIAGM = 108
B_PCF = 172
B_PTR = 178           # row 0: ptr2a, ptr2b, ptr3a, ptr3b (4 x 384)
MISCBW = 1714

_COMPILED = {}


# ----------------------------------------------------------------------------
# host-side prep
# ----------------------------------------------------------------------------

def _host_prep(src_x, tgt_x, src_y, tgt_y):
    import ml_dtypes
    bf16 = ml_dtypes.bfloat16
    src_x = np.ascontiguousarray(np.asarray(src_x, dtype=np.float32))
    tgt_x = np.ascontiguousarray(np.asarray(tgt_x, dtype=np.float32))
    src_y = np.asarray(src_y).astype(np.int64)
    pt = np.ascontiguousarray(np.asarray(tgt_y, dtype=np.float32))

    counts = np.bincount(src_y, minlength=C)
    if counts.max() > CAP:
        return None  # caller falls back to numpy path

    perm = np.argsort(src_y, kind="stable")
    sx_pad = np.zeros((R, D), np.float32)
    W = np.zeros((R, C), np.float32)
    # pad sentinels: huge random-sign vectors. Pad-pad dot products are then
    # tiny relative to the norms (no catastrophic cancellation in d2), every
    # pad-involved distance is >= ~3e5 and exp(-dist/bw) underflows to 0.
    rng = np.random.default_rng(987654321)
    sgn = (rng.integers(0, 2, size=(R, D)).astype(np.float32) * 2.0 - 1.0)
    off = 0
    padidx = 0
    padrow = np.zeros(R, bool)
    for c in range(C):
        idx = perm[off:off + counts[c]]
        sx_pad[c * CAP:c * CAP + counts[c]] = src_x[idx]
        W[c * CAP:c * CAP + counts[c], c] = 1.0
        padrow[c * CAP + counts[c]:(c + 1) * CAP] = True
        for p in range(CAP - counts[c]):
            sx_pad[c * CAP + counts[c] + p, :] = 2.0e4 * sgn[padidx]
            padidx += 1
        off += counts[c]
    # per-row d2 epsilon: pad rows have ~1e11 norms where a 0.5 epsilon
    # vanishes in f32 and accumulation noise could push d2 negative (no
    # clamp on device); a 1e9 floor keeps sqrt safe and only perturbs
    # pad distances, whose exp underflows to 0 regardless
    eps_row = np.where(padrow, 1.0e9, EPS).astype(np.float32)

    # round features to bf16 host-side; norms are computed from the rounded
    # values in f32 so the d2 diagonal cancels to ~0 on device
    sx_bf = sx_pad.astype(bf16)
    tx_bf = tgt_x.astype(bf16)
    sx_rf = sx_bf.astype(np.float32)
    tx_rf = tx_bf.astype(np.float32)

    txT_pack = np.zeros((128, 768), bf16)
    for k in range(2):
        txT_pack[:, k * N:(k + 1) * N] = tx_bf.T[k * 128:(k + 1) * 128, :]
    rtcol3 = np.zeros((128, 3), np.float32)
    for blk in range(3):
        rtcol3[:, blk] = (tx_rf[blk * 128:(blk + 1) * 128] ** 2).sum(1) + EPS
    rtrow = (tx_rf ** 2).sum(1)

    cs = counts.astype(np.float64)
    ct = pt.sum(0).astype(np.float64)
    pss = cs * cs
    ptt = ct * ct

    rden2 = (1.0 / (pss[:, None] + pss[None, :]
                    + 2.0 * cs[:, None] * cs[None, :])).astype(np.float32)
    rdenin = (1.0 / (pss + ptt + 2.0 * cs * ct)).astype(np.float32).reshape(C, 1)

    pw5 = np.array([-(float(MU) ** (k - KN // 2)) for k in range(KN)],
                   np.float32)
    pw60 = np.zeros((C, 60), np.float32)
    for k in range(KN):
        pw60[:, k * 12:(k + 1) * 12] = pw5[k]

    in_maps = []
    for r in range(NCORES):
        g = r % 6
        a, b = 2 * g, 2 * g + 1
        pp_active = r < 6
        roll = 2 * g * CAP

        sxr_bf = np.roll(sx_bf, -roll, axis=0)
        sxr_rf = np.roll(sx_rf, -roll, axis=0)
        sxT_pack = np.zeros((128, 1536), bf16)
        for k in range(2):
            sxT_pack[:, k * R:(k + 1) * R] = sxr_bf.T[k * 128:(k + 1) * 128, :]
        norms = (sxr_rf ** 2).sum(1)
        eps_r = np.roll(eps_row, -roll)
        rscol6 = (norms + eps_r).reshape(6, 128).T
        rsrow = norms

        wr = np.roll(W, -roll, axis=0)

        oh2 = np.zeros((C, 2), np.float32)
        oh2[a, 0] = 1.0
        oh2[b, 1] = 1.0

        # k2 split: q0 = class r with all 5 bandwidths; q1 = class 8+(r%4)
        # with bandwidths {0,1,2} on cores 0-3 and {3,4,dup} on cores 4-7.
        c_q0 = r
        c_q1 = 8 + (r % 4)
        kq1 = [0, 1, 2] if r < 4 else [3, 4]
        k2sel = np.zeros((C, 2), np.float32)
        k2sel[c_q0, 0] = 1.0
        k2sel[c_q1, 1] = 1.0
        pw10 = np.zeros((C, 10), np.float32)
        pw10[:, 0:5] = pw5[None, :]
        for j in range(5):
            pw10[:, 5 + j] = pw5[kq1[j]] if j < len(kq1) else pw5[0]

        ptrow2 = np.zeros((2, N), np.float32)
        ptcolf = np.zeros((128, 6), np.float32)
        for q, c in enumerate((c_q0, c_q1)):
            ptrow2[q] = pt[:, c]
            for blk in range(3):
                ptcolf[:, q * 3 + blk] = pt[blk * 128:(blk + 1) * 128, c]

        # perm65: sclT3[j] = ibg[cls, perm(j)] via matmul(lhsT=perm65, rhs=selcol)
        # row layout j = k*12 + t; source col = k*12 + rot(t), rot(t)=(2g+t)%12
        perm65 = np.zeros((65, 65), np.float32)
        for k in range(KN):
            for t in range(12):
                perm65[k * 12 + ((2 * g + t) % 12), k * 12 + t] = 1.0
        for j in range(60, 65):
            perm65[j, j] = 1.0

        wm = np.zeros((128, NCOL), np.float32)
        if pp_active:
            for h, cls in ((0, a), (1, b)):
                for k in range(KN):
                    for t in range(12):
                        if t != cls:
                            wm[h * 64 + k * 12 + t, 0] = I2 / pss[cls]
                        rt_ = (2 * g + t) % 12
                        if rt_ != cls:
                            wm[h * 64 + k * 12 + t, 1] = \
                                -I2 / (cs[cls] * cs[rt_])
                for k in range(KN):
                    wm[h * CAP:(h + 1) * CAP, 2 + k] = 1.0 / (C * pss[cls])
                    wm[h * CAP:(h + 1) * CAP, 7 + k] = \
                        -2.0 / (C * cs[cls] * ct[cls])
        wm[:, 12:17] = 1.0 / (C * ptt[c_q0])
        for j in range(len(kq1)):
            wm[:, 17 + j] = 1.0 / (C * ptt[c_q1])

        ssel = np.zeros((NCOL, 2), np.float32)
        ssel[2:NCOL, 0] = 1.0   # intra cols: k1, k3, k2
        ssel[0:2, 1] = 1.0      # inter cols: T1, T3

        offs = np.zeros((1, 2), np.float32)
        if r == 0:
            corr = 5.0 * CAP - DIAG5 * cs
            offs[0, 0] = -(corr / pss / C).sum()
            offs[0, 1] = -((C - 1) * corr * I2 / pss).sum()

        misc = np.zeros((128, MISCW), np.float32)
        misc[:, O_WM:O_WM + NCOL] = wm
        misc[:, O_ONES] = 1.0
        misc[0, O_ONESR:O_ONESR + 128] = 1.0
        misc[:, O_RSCOL6:O_RSCOL6 + 6] = rscol6
        misc[:, O_RTCOL:O_RTCOL + 3] = rtcol3
        misc[0, O_RSROW:O_RSROW + R] = rsrow
        misc[0, O_RTROW:O_RTROW + N] = rtrow
        misc[0:12, O_EYE12:O_EYE12 + 12] = np.eye(C, dtype=np.float32)
        misc[0:12, O_OH2:O_OH2 + 2] = oh2
        misc[0:12, O_K2SEL:O_K2SEL + 2] = k2sel
        misc[0:65, O_PERM65:O_PERM65 + 65] = perm65
        misc[0:12, O_PW60:O_PW60 + 60] = pw60
        misc[0:12, O_PW10:O_PW10 + 10] = pw10
        misc[0:12, O_RDEN2:O_RDEN2 + 12] = rden2
        misc[0:12, O_RDENIN:O_RDENIN + 1] = rdenin
        misc[0:NCOL, O_SSEL:O_SSEL + 2] = ssel
        misc[0, O_OFFS:O_OFFS + 2] = offs[0]

        miscb = np.zeros((128, MISCBW), np.float32)
        for m in range(6):
            miscb[:, B_WR + m * 12:B_WR + (m + 1) * 12] = \
                wr[m * 128:(m + 1) * 128]
        for m in range(3):
            miscb[:, B_PTB + m * 12:B_PTB + (m + 1) * 12] = \
                pt[m * 128:(m + 1) * 128]
        miscb[0:CAP, B_DIAGM:B_DIAGM + CAP] = 1.0 - np.eye(CAP)
        miscb[CAP:128, B_DIAGM:B_DIAGM + CAP] = 1.0 - np.eye(CAP)
        miscb[:, B_PCF:B_PCF + 6] = ptcolf
        miscb[0, B_PTR:B_PTR + N] = ptrow2[0]
        miscb[0, B_PTR + N:B_PTR + 2 * N] = ptrow2[1]
        miscb[0, B_PTR + 2 * N:B_PTR + 3 * N] = pt[:, a]
        miscb[0, B_PTR + 3 * N:B_PTR + 4 * N] = pt[:, b]

        in_maps.append({
            "sxT": sxT_pack,
            "txT": txT_pack,
            "misc": np.ascontiguousarray(misc),
            "miscb": np.ascontiguousarray(miscb.astype(bf16)),
        })
    return in_maps


def _numpy_fallback(src_x, tgt_x, src_y, tgt_y):
    f = np.float32
    src_x = np.asarray(src_x, f)
    tgt_x = np.asarray(tgt_x, f)
    src_y = np.asarray(src_y).astype(np.int64)
    pt = np.asarray(tgt_y, f)
    ps = np.eye(C, dtype=f)[src_y]

    def cdist(a, bb):
        d2 = (a * a).sum(1)[:, None] + (bb * bb).sum(1)[None, :] - 2.0 * (a @ bb.T)
        return np.sqrt(np.maximum(d2, 0.0))

    def kern(dist, g):
        acc = 0.0
        for i in range(KN):
            bw = np.maximum(np.asarray(g) * (MU ** (i - KN // 2)), 1e-5)
            acc = acc + np.exp(-np.clip(dist / bw, 1e-5, 1e5))
        return acc

    E_ss = cdist(src_x, src_x); E_tt = cdist(tgt_x, tgt_x); E_st = cdist(src_x, tgt_x)
    sss = np.einsum('ic,ij,jc->c', ps, E_ss, ps)
    stt = np.einsum('ic,ij,jc->c', pt, E_tt, pt)
    sst = np.einsum('is,ij,jt->st', ps, E_st, pt)
    cs = ps.sum(0); ct = pt.sum(0)
    pss = cs * cs; ptt = ct * ct; pstd = cs * ct
    g_in = (sss + stt + 2 * np.diagonal(sst)) / (pss + ptt + 2 * pstd)
    Pss = ps.T[:, :, None] * ps.T[:, None, :]
    Ptt = pt.T[:, :, None] * pt.T[:, None, :]
    Pst = ps.T[:, :, None] * pt.T[:, None, :]
    k1 = (kern(E_ss[None] * Pss, g_in[:, None, None]) * Pss).sum((-2, -1)) / pss
    k2 = (kern(E_tt[None] * Ptt, g_in[:, None, None]) * Ptt).sum((-2, -1)) / ptt
    k3 = (kern(E_st[None] * Pst, g_in[:, None, None]) * Pst).sum((-2, -1)) / pstd
    intra = (k1 + k2 - 2 * k3).sum() / C
    sst_s = np.einsum('is,ij,jt->st', ps, E_ss, ps)
    g2 = (sss[:, None] + sss[None, :] + 2 * sst_s) / (
        pss[:, None] + pss[None, :] + 2 * cs[:, None] * cs[None, :])
    T1 = np.zeros((C, C), f); T3 = np.zeros((C, C), f)
    for s in range(C):
        ms = ps[:, s].astype(bool)
        for t in range(C):
            mt = ps[:, t].astype(bool)
            T1[s, t] = kern(E_ss[np.ix_(ms, ms)], g2[s, t]).sum() / pss[s]
            T3[s, t] = kern(E_ss[np.ix_(ms, mt)], g2[s, t]).sum() / (cs[s] * cs[t])
    inter = ((2 * T1 - 2 * T3) * (1 - np.eye(C))).sum() / (C * C - C)
    return np.array([intra, inter], np.float32)


# ----------------------------------------------------------------------------
# device program
# ----------------------------------------------------------------------------

def _build_program():
    import concourse.bass as bass
    import concourse.tile as tile
    from concourse import bacc, mybir

    f32 = mybir.dt.float32
    bf = mybir.dt.bfloat16
    AF = mybir.ActivationFunctionType
    OP = mybir.AluOpType

    nc = bacc.Bacc("TRN2", target_bir_lowering=False, debug=False,
                   num_devices=NCORES)

    i_sxT = nc.dram_tensor("sxT", [128, 2 * R], bf, kind="ExternalInput").ap()
    i_txT = nc.dram_tensor("txT", [128, 2 * N], bf, kind="ExternalInput").ap()
    i_misc = nc.dram_tensor("misc", [128, MISCW], f32, kind="ExternalInput").ap()
    i_miscb = nc.dram_tensor("miscb", [128, MISCBW], bf,
                             kind="ExternalInput").ap()

    o_out = nc.dram_tensor("out", [1, 2], f32, kind="ExternalOutput").ap()

    with tile.TileContext(nc) as tc:
        with (
            tc.tile_pool(name="io", bufs=1) as io,
            tc.tile_pool(name="big", bufs=1) as big,
            tc.tile_pool(name="scr", bufs=2) as scr,
            tc.tile_pool(name="sm", bufs=1) as sm,
            tc.tile_pool(name="pG", bufs=2, space="PSUM") as pG,
            tc.tile_pool(name="p1", bufs=2, space="PSUM") as p1,
            tc.tile_pool(name="pT", bufs=2, space="PSUM") as pT,
            tc.tile_pool(name="pS", bufs=1, space="PSUM") as pS,
            tc.tile_pool(name="dram", bufs=1, space="DRAM") as dpool,
        ):
            dma_sp = nc.sync.dma_start
            dma_act = nc.scalar.dma_start

            # ---------------- input loads: 4 big DMAs ----------------
            sxT = io.tile([128, 2 * R], bf, tag="sxT", name="sxT")
            dma_sp(out=sxT[:], in_=i_sxT[:])
            txT = io.tile([128, 2 * N], bf, tag="txT", name="txT")
            dma_act(out=txT[:], in_=i_txT[:])
            misc = io.tile([128, MISCW], f32, tag="misc", name="misc")
            dma_sp(out=misc[:], in_=i_misc[:])
            miscb = io.tile([128, MISCBW], bf, tag="miscb", name="miscb")
            dma_act(out=miscb[:], in_=i_miscb[:])

            wm = misc[:, O_WM:O_WM + NCOL]
            ones = misc[:, O_ONES:O_ONES + 1]
            onesr = misc[0:1, O_ONESR:O_ONESR + 128]
            rscol6 = misc[:, O_RSCOL6:O_RSCOL6 + 6]
            rtcol = misc[:, O_RTCOL:O_RTCOL + 3]
            rsrow = misc[0:1, O_RSROW:O_RSROW + R]
            rtrow = misc[0:1, O_RTROW:O_RTROW + N]
            eye12 = misc[0:12, O_EYE12:O_EYE12 + 12]
            oh2 = misc[0:12, O_OH2:O_OH2 + 2]
            k2sel = misc[0:12, O_K2SEL:O_K2SEL + 2]
            perm65 = misc[0:65, O_PERM65:O_PERM65 + 65]
            pw60 = misc[0:12, O_PW60:O_PW60 + 60]
            pw10 = misc[0:12, O_PW10:O_PW10 + 10]
            rden2 = misc[0:12, O_RDEN2:O_RDEN2 + 12]
            rdenin = misc[0:12, O_RDENIN:O_RDENIN + 1]
            ssel = misc[0:NCOL, O_SSEL:O_SSEL + 2]
            offs = misc[0:1, O_OFFS:O_OFFS + 2]

            wrb = [miscb[:, B_WR + m * 12:B_WR + (m + 1) * 12] for m in range(6)]
            ptb = [miscb[:, B_PTB + m * 12:B_PTB + (m + 1) * 12]
                   for m in range(3)]
            diagm = miscb[:, B_DIAGM:B_DIAGM + CAP]
            pcf = miscb[:, B_PCF:B_PCF + 6]
            ptr2 = [miscb[0:1, B_PTR + q * N:B_PTR + (q + 1) * N]
                    for q in range(2)]
            ptr3 = [miscb[0:1, B_PTR + (q + 2) * N:B_PTR + (q + 3) * N]
                    for q in range(2)]

            # row-norm broadcasts via rank-1 TensorE matmul (ones x row):
            # gpsimd has multi-us startup latency and would gate the whole
            # E chain; TensorE is idle here
            rsrowb = big.tile([128, R], f32, tag="rsrowb", name="rsrowb")
            rtrowb = big.tile([128, N], f32, tag="rtrowb", name="rtrowb")
            for lo, w in ((0, 512), (512, 256)):
                pb = pG.tile([128, 512], f32, tag="G", name="pb")
                nc.tensor.matmul(pb[:, :w], onesr, rsrow[0:1, lo:lo + w],
                                 start=True, stop=True)
                nc.vector.tensor_copy(rsrowb[:, lo:lo + w], pb[:, :w])
            pb = pG.tile([128, 512], f32, tag="G", name="pb")
            nc.tensor.matmul(pb[:, :N], onesr, rtrow, start=True, stop=True)
            nc.vector.tensor_copy(rtrowb[:], pb[:, :N])

            # ---------------- E matrices (bf16 in/out, f32 d2) ----------------
            sxTk = [sxT[:, 0:R], sxT[:, R:2 * R]]
            txTk = [txT[:, 0:N], txT[:, N:2 * N]]

            def emit_E(dst, lhsT_k, lhs_lo, rhs_k, n_cols, rcol_ap, rowb,
                       rhs_lo=0):
                done = 0
                while done < n_cols:
                    nchunk = min(512, n_cols - done)
                    gp = pG.tile([128, 512], f32, tag="G", name="gp")
                    for k in range(2):
                        nc.tensor.matmul(
                            gp[:, :nchunk],
                            lhsT_k[k][:, lhs_lo:lhs_lo + 128],
                            rhs_k[k][:, rhs_lo + done:rhs_lo + done + nchunk],
                            start=(k == 0), stop=(k == 1))
                    t1_ = scr.tile([128, 512], f32, tag="d2scr", name="d2s")
                    nc.vector.scalar_tensor_tensor(
                        out=t1_[:, :nchunk], in0=gp[:, :nchunk], scalar=-2.0,
                        in1=rowb[:, rhs_lo + done:rhs_lo + done + nchunk],
                        op0=OP.mult, op1=OP.add)
                    nc.scalar.activation(dst[:, done:done + nchunk],
                                         t1_[:, :nchunk], AF.Sqrt,
                                         bias=rcol_ap)
                    done += nchunk

            # own slab first (feeds the T1/T3 gathers), then E_tt (k2 builds),
            # then E_pt, then the upper-triangle blocks of the remaining
            # E_ss slabs (S1 is recovered as U + U^T - blockdiag)
            E_ss = [big.tile([128, R - 128 * s], bf, tag=f"E_ss{s}",
                             name=f"E_ss{s}") for s in range(6)]
            E_own = E_ss[0]
            emit_E(E_own, sxTk, 0, sxTk, R, rscol6[:, 0:1], rsrowb)

            E_ttf = big.tile([128, 3 * N], bf, tag="E_ttf", name="E_ttf")
            for blk in range(3):
                emit_E(E_ttf[:, blk * N:(blk + 1) * N], txTk, blk * 128, txTk,
                       N, rtcol[:, blk:blk + 1], rtrowb)

            E_ptf = big.tile([128, 6 * N], bf, tag="E_ptf", name="E_ptf")
            for s in range(6):
                emit_E(E_ptf[:, s * N:(s + 1) * N], sxTk, s * 128, txTk, N,
                       rscol6[:, s:s + 1], rtrowb)

            for s in range(1, 6):
                emit_E(E_ss[s], sxTk, s * 128, sxTk, R - 128 * s,
                       rscol6[:, s:s + 1], rsrowb, rhs_lo=128 * s)

            # ---------------- local global sums (no collective) --------------
            part = sm.tile([C, 36], f32, tag="part", name="part")

            # S1 = U + U^T - BD with U = wr^T E_upper wr (E_ss[s] holds cols
            # s*128..768, i.e. upper blocks incl. the diagonal block)
            ups = pS.tile([C, C], f32, tag="S", name="ups")
            for sub in range(6):
                pp = p1.tile([128, C], f32, tag="p1", name="pp")
                for slab in range(sub + 1):
                    nc.tensor.matmul(
                        pp[:],
                        E_ss[slab][:, (sub - slab) * 128:(sub - slab + 1) * 128],
                        wrb[slab], start=(slab == 0), stop=(slab == sub))
                cb = scr.tile([128, C], bf, tag="cbs", name="cb")
                nc.vector.tensor_copy(cb[:], pp[:])
                nc.tensor.matmul(ups[:], cb[:], wrb[sub],
                                 start=(sub == 0), stop=(sub == 5))
            usb = sm.tile([C, C], f32, tag="usb", name="usb")
            nc.vector.tensor_copy(usb[:], ups[:])

            bdps = pS.tile([C, C], f32, tag="S", name="bdps")
            for sub in range(6):
                pp = p1.tile([128, C], f32, tag="p1", name="pp")
                nc.tensor.matmul(pp[:], E_ss[sub][:, 0:128], wrb[sub],
                                 start=True, stop=True)
                cb = scr.tile([128, C], bf, tag="cbs", name="cb")
                nc.vector.tensor_copy(cb[:], pp[:])
                nc.tensor.matmul(bdps[:], cb[:], wrb[sub],
                                 start=(sub == 0), stop=(sub == 5))
            bdsb = sm.tile([C, C], f32, tag="bdsb", name="bdsb")
            nc.vector.tensor_copy(bdsb[:], bdps[:])

            utp = pT.tile([C, C], f32, tag="tiny", name="utp")
            nc.tensor.transpose(utp[:], usb[:], eye12)
            nc.vector.tensor_tensor(part[:, 0:12], usb[:], utp[:], OP.add)
            nc.vector.tensor_tensor(part[:, 0:12], part[:, 0:12], bdsb[:],
                                    OP.subtract)

            stps = pS.tile([C, C], f32, tag="S", name="stps")
            for sub in range(3):
                pp = p1.tile([128, C], f32, tag="p1", name="pp")
                for blk in range(3):
                    nc.tensor.matmul(
                        pp[:],
                        E_ttf[:, blk * N + sub * 128:blk * N + (sub + 1) * 128],
                        ptb[blk], start=(blk == 0), stop=(blk == 2))
                cb = scr.tile([128, C], bf, tag="cbs", name="cb")
                nc.vector.tensor_copy(cb[:], pp[:])
                nc.tensor.matmul(stps[:], cb[:], ptb[sub],
                                 start=(sub == 0), stop=(sub == 2))
            nc.vector.tensor_copy(part[:, 12:24], stps[:])

            ssps = pS.tile([C, C], f32, tag="S", name="ssps")
            for sub in range(3):
                pp = p1.tile([128, C], f32, tag="p1", name="pp")
                for slab in range(6):
                    nc.tensor.matmul(
                        pp[:],
                        E_ptf[:, slab * N + sub * 128:slab * N + (sub + 1) * 128],
                        wrb[slab], start=(slab == 0), stop=(slab == 5))
                cb = scr.tile([128, C], bf, tag="cbs", name="cb")
                nc.vector.tensor_copy(cb[:], pp[:])
                nc.tensor.matmul(ssps[:], cb[:], ptb[sub],
                                 start=(sub == 0), stop=(sub == 2))
            nc.vector.tensor_copy(part[:, 24:36], ssps[:])

            # ---------------- T1/T3 gathers + k2/k3 builds ----------------
            E_diag = big.tile([128, CAP], bf, tag="E_diag", name="E_diag")
            nc.vector.tensor_tensor(E_diag[0:CAP, :], E_own[0:CAP, 0:CAP],
                                    diagm[0:CAP, :], OP.mult)
            nc.vector.tensor_tensor(E_diag[CAP:128, :],
                                    E_own[CAP:128, CAP:128],
                                    diagm[CAP:128, :], OP.mult)

            d_ed = dpool.tile([128, CAP], bf, tag="d_ed", name="d_ed")
            dma_sp(out=d_ed[:], in_=E_diag[:])
            # E_own -> DRAM in class-block layout [t][row][col] so each
            # (half, t) block is one contiguous segment
            d_eob = dpool.tile([C, 128 * CAP], bf, tag="d_eob", name="d_eob")
            dst_eob = bass.AP(tensor=d_eob.tensor, offset=0,
                              ap=[[CAP, 128], [128 * CAP, 12], [1, CAP]])
            dma_act(out=dst_eob, in_=E_own[:])

            t1src = big.tile([128, CAP * CAP], bf, tag="t1src", name="t1src")
            for h in range(2):
                ap_in = bass.AP(tensor=d_ed.tensor, offset=h * CAP * CAP,
                                ap=[[0, 64], [1, CAP * CAP]])
                dma_sp(out=t1src[h * 64:(h + 1) * 64, :], in_=ap_in)

            # T3 flat blocks: partition h*64 + k*12 + t <- block (h, t)
            t3src = big.tile([128, CAP * CAP], bf, tag="t3src", name="t3src")
            for h in range(2):
                ap_in = bass.AP(tensor=d_eob.tensor, offset=h * CAP * CAP,
                                ap=[[0, 5], [128 * CAP, 12], [1, CAP * CAP]])
                dma_act(out=t3src[h * 64:h * 64 + 60, :], in_=ap_in)
                ap_pad = bass.AP(tensor=d_eob.tensor, offset=h * CAP * CAP,
                                 ap=[[0, 4], [1, CAP * CAP]])
                dma_act(out=t3src[h * 64 + 60:h * 64 + 64, :], in_=ap_pad)

            # dummy Exp to pull the ACT table load off the critical path
            # (runs right after the last sqrt, while the gammas compute)
            dmy = scr.tile([1, 4], bf, tag="dmy", name="dmy")
            nc.scalar.activation(dmy[:], E_diag[0:1, 0:4], AF.Exp)

            # late gpsimd broadcasts (after the E-chain Vector work)
            ptrow2b = [big.tile([128, N], bf, tag=f"ptrow2b{q}",
                                name=f"ptrow2b{q}") for q in range(2)]
            nc.gpsimd.partition_broadcast(ptrow2b[0][:], ptr2[0])
            nc.gpsimd.partition_broadcast(ptrow2b[1][:], ptr2[1])
            ptw3 = big.tile([128, N], bf, tag="ptw3", name="ptw3")
            ptw3t = big.tile([128, N], bf, tag="ptw3t", name="ptw3t")
            nc.gpsimd.partition_broadcast(ptw3[:], ptr3[0])
            nc.gpsimd.partition_broadcast(ptw3t[:], ptr3[1])
            nc.vector.tensor_copy(ptw3[CAP:128, :], ptw3t[CAP:128, :])

            k2P = []
            k2D = []
            for q in range(2):
                P = big.tile([128, 3 * N], bf, tag=f"k2P{q}", name=f"k2P{q}")
                colap = bass.AP(tensor=pcf.tensor,
                                offset=pcf.offset + q * 3,
                                ap=[list(pcf.ap[0]), [1, 3], [0, N]])
                rowap = bass.AP(tensor=ptrow2b[q].tensor,
                                offset=ptrow2b[q].offset,
                                ap=[list(ptrow2b[q].ap[0]), [0, 3], [1, N]])
                nc.vector.tensor_tensor(P[:], colap, rowap, OP.mult)
                Dt = big.tile([128, 3 * N], bf, tag=f"k2D{q}", name=f"k2D{q}")
                nc.vector.tensor_tensor(Dt[:], E_ttf[:], P[:], OP.mult)
                k2P.append(P)
                k2D.append(Dt)

            k3D = big.tile([128, N], bf, tag="k3D", name="k3D")
            nc.vector.tensor_tensor(k3D[:], E_ptf[:, 0:N], ptw3[:], OP.mult)

            acc = big.tile([128, NCOL], f32, tag="acc", name="acc")
            nc.vector.memset(acc[:], 0.0)
            sclT1 = sm.tile([128, 1], f32, tag="sclT1", name="sclT1")
            sclT3 = sm.tile([128, 1], f32, tag="sclT3", name="sclT3")
            nc.vector.memset(sclT1[:], 0.0)
            nc.vector.memset(sclT3[:], 0.0)

            # ---------------- gammas (from the local sums) ----------------
            S1 = part[:, 0:12]
            sttM = part[:, 12:24]
            sstM = part[:, 24:36]

            def diag_col(mat, nm):
                s_ = scr.tile([C, C], f32, tag="diagscr", name="dsc")
                col = sm.tile([C, 1], f32, tag=nm, name=nm)
                nc.vector.tensor_tensor(s_[:], mat, eye12, OP.mult)
                nc.vector.reduce_sum(out=col[:], in_=s_[:],
                                     axis=mybir.AxisListType.X)
                return col

            ssscol = diag_col(S1, "ssscol")
            sttcol = diag_col(sttM, "sttcol")
            sstdcol = diag_col(sstM, "sstdcol")

            gin = sm.tile([C, 1], f32, tag="gin", name="gin")
            nc.vector.scalar_tensor_tensor(out=gin[:], in0=sstdcol[:], scalar=2.0,
                                           in1=sttcol[:], op0=OP.mult, op1=OP.add)
            nc.vector.tensor_tensor(gin[:], gin[:], ssscol[:], OP.add)
            nc.vector.tensor_tensor(gin[:], gin[:], rdenin, OP.mult)

            ssst = pT.tile([1, C], f32, tag="tiny", name="ssst")
            nc.tensor.transpose(ssst[:], ssscol[:], eye12)
            ssstsb = sm.tile([1, C], f32, tag="ssstsb", name="ssstsb")
            nc.vector.tensor_copy(ssstsb[:], ssst[:])
            ps12 = pT.tile([C, C], f32, tag="tiny", name="ps12")
            nc.tensor.matmul(ps12[:], onesr[0:1, 0:12], ssstsb[:],
                             start=True, stop=True)
            sssrowb = sm.tile([C, C], f32, tag="sssrowb", name="sssrowb")
            nc.vector.tensor_copy(sssrowb[:], ps12[:])
            g2 = sm.tile([C, C], f32, tag="g2", name="g2")
            nc.vector.tensor_scalar(g2[:], S1, 2.0, None, OP.mult)
            nc.vector.tensor_tensor(g2[:], g2[:], sssrowb[:], OP.add)
            nc.vector.tensor_scalar(g2[:], g2[:], ssscol[:], None, OP.add)
            nc.vector.tensor_tensor(g2[:], g2[:], rden2, OP.mult)

            # IBG [12, 70] = -1/bw : cols 0-59 from g2 (k-major),
            # 60-64 from gin (k-order), 65-69 from gin (host-permuted for q1)
            ibg0 = sm.tile([C, 70], f32, tag="ibg0", name="ibg0")
            g2ap = g2[:]
            g2exp = bass.AP(tensor=g2ap.tensor, offset=g2ap.offset,
                            ap=[list(g2ap.ap[0]), [0, 5], [1, 12]])
            nc.vector.tensor_tensor(ibg0[:, 0:60], g2exp, pw60, OP.mult)
            ginap = gin[:]
            ginexp = bass.AP(tensor=ginap.tensor, offset=ginap.offset,
                             ap=[list(ginap.ap[0]), [0, 10]])
            nc.vector.tensor_tensor(ibg0[:, 60:70], ginexp, pw10, OP.mult)
            nc.vector.tensor_scalar(ibg0[:], ibg0[:], -1e-5, None, OP.min)
            ibg = sm.tile([C, 70], f32, tag="ibg", name="ibg")
            nc.vector.reciprocal(ibg[:], ibg0[:])

            # per-half scale vectors
            negk1 = sm.tile([128, 5], f32, tag="negk1", name="negk1")
            for h in range(2):
                ps_ = pT.tile([1, 65], f32, tag="tiny", name="psel")
                nc.tensor.matmul(ps_[:], oh2[:, h:h + 1], ibg[:, 0:65],
                                 start=True, stop=True)
                selsb = sm.tile([1, 65], f32, tag=f"sel{h}", name=f"sel{h}")
                nc.vector.tensor_copy(selsb[:], ps_[:])
                tp_ = pT.tile([65, 1], f32, tag="tiny", name="tsel")
                nc.tensor.transpose(tp_[:], selsb[:], ones[0:1, :])
                tpsb = scr.tile([65, 1], f32, tag="tselsb", name="tpsb")
                nc.vector.tensor_copy(tpsb[:], tp_[:])
                nc.vector.tensor_copy(sclT1[h * 64:h * 64 + 60, :], tpsb[0:60, :])
                p3 = pT.tile([65, 1], f32, tag="tiny", name="p3")
                nc.tensor.matmul(p3[:], perm65, tpsb[:], start=True, stop=True)
                p3sb = scr.tile([65, 1], f32, tag="p3sb", name="p3sb")
                nc.vector.tensor_copy(p3sb[:], p3[:])
                nc.vector.tensor_copy(sclT3[h * 64:h * 64 + 60, :], p3sb[0:60, :])
                pnk = pT.tile([128, 5], f32, tag="tiny", name="pnk")
                nc.tensor.matmul(pnk[:], onesr, selsb[0:1, 60:65],
                                 start=True, stop=True)
                if h == 0:
                    nc.vector.tensor_copy(negk1[0:CAP, :], pnk[0:CAP, :])
                else:
                    nc.vector.tensor_copy(negk1[CAP:128, :], pnk[CAP:128, :])

            negb = []
            for q in range(2):
                k2sc = pT.tile([1, 5], f32, tag="tiny", name="k2sc")
                nc.tensor.matmul(k2sc[:], k2sel[:, q:q + 1],
                                 ibg[:, 60 + 5 * q:65 + 5 * q],
                                 start=True, stop=True)
                k2scsb = sm.tile([1, 5], f32, tag=f"k2scsb{q}", name=f"k2scsb{q}")
                nc.vector.tensor_copy(k2scsb[:], k2sc[:])
                pnb = pT.tile([128, 5], f32, tag="tiny", name="pnb")
                nc.tensor.matmul(pnb[:], onesr, k2scsb[:], start=True, stop=True)
                nb = sm.tile([128, 5], f32, tag=f"negb{q}", name=f"negb{q}")
                nc.vector.tensor_copy(nb[:], pnb[:])
                negb.append(nb)

            # ---------------- exp passes ----------------
            nc.scalar.activation(t1src[:], t1src[:], AF.Exp, scale=sclT1[:],
                                 accum_out=acc[:, 0:1])
            nc.scalar.activation(t3src[:], t3src[:], AF.Exp, scale=sclT3[:],
                                 accum_out=acc[:, 1:2])

            for k in range(KN):
                sk = scr.tile([128, CAP], bf, tag="k1scr", name="sk1")
                nc.scalar.activation(sk[:], E_diag[:], AF.Exp,
                                     scale=negk1[:, k:k + 1],
                                     accum_out=acc[:, 2 + k:3 + k])

            for k in range(KN):
                ek = scr.tile([128, N], bf, tag="k3e", name="ek3")
                nc.scalar.activation(ek[:], k3D[:], AF.Exp,
                                     scale=negk1[:, k:k + 1])
                sk = scr.tile([128, N], bf, tag="k3scr", name="sk3")
                nc.vector.scalar_tensor_tensor(
                    out=sk[:], in0=ek[:], scalar=1.0, in1=ptw3[:],
                    op0=OP.mult, op1=OP.mult,
                    accum_out=acc[:, 7 + k:8 + k])

            for q in range(2):
                npass = 5 if q == 0 else 3
                for j in range(npass):
                    ek = scr.tile([128, 3 * N], bf, tag="k2e", name="ek2")
                    nc.scalar.activation(ek[:], k2D[q][:], AF.Exp,
                                         scale=negb[q][:, j:j + 1])
                    sk = scr.tile([128, 3 * N], bf, tag="k2scr", name="sk2")
                    col = 12 + 5 * q + j
                    nc.vector.scalar_tensor_tensor(
                        out=sk[:], in0=ek[:], scalar=1.0, in1=k2P[q][:],
                        op0=OP.mult, op1=OP.mult,
                        accum_out=acc[:, col:col + 1])

            # ---------------- final weighted reduce ----------------
            v = big.tile([128, NCOL], f32, tag="v", name="v")
            nc.vector.tensor_tensor(v[:], acc[:], wm, OP.mult)
            m1 = pT.tile([NCOL, 1], f32, tag="tiny", name="m1")
            nc.tensor.matmul(m1[:], v[:], ones, start=True, stop=True)
            m1sb = sm.tile([NCOL, 1], f32, tag="m1sb", name="m1sb")
            nc.vector.tensor_copy(m1sb[:], m1[:])
            m2 = pT.tile([1, 2], f32, tag="tiny", name="m2")
            nc.tensor.matmul(m2[:], m1sb[:], ssel, start=True, stop=True)
            res = sm.tile([1, 2], f32, tag="res", name="res")
            nc.vector.tensor_tensor(res[:], m2[:], offs, OP.add)
            dma_sp(out=o_out[:], in_=res[:])

    nc.compile()
    return nc


def get_program():
    if "nc" not in _COMPILED:
        _COMPILED["nc"] = _build_program()
    return _COMPILED["nc"]


# ----------------------------------------------------------------------------
# entry point
# ----------------------------------------------------------------------------

def _run(in_maps, trace=False):
    from concourse.bass_utils import run_bass_kernel_spmd
    nc = get_program()
    return run_bass_kernel_spmd(nc, in_maps, list(range(NCORES)), trace=trace)


def kernel(src_x, tgt_x, src_y, tgt_y):
    in_maps = _host_prep(src_x, tgt_x, src_y, tgt_y)
    if in_maps is None:
        return _numpy_fallback(src_x, tgt_x, src_y, tgt_y)
    br = _run(in_maps)
    total = np.zeros(2, np.float64)
    for res in br.results:
        total += res["out"].reshape(2).astype(np.float64)
    return total.astype(np.float32)



# revision 18
# speedup vs baseline: 1.3042x; 1.3042x over previous
"""CDD loss kernel for 8 Trainium2 NeuronCores (Bass/Tile, SPMD).

Math (validated vs reference in float32):
  ps is one-hot -> every (C,C,N,N) reference tensor collapses to per-class-
  block sums. Host sorts+pads src rows by class (CAP rows/class, pads are
  huge distinct sentinel vectors so exp(-dist/bw) underflows to exactly 0).
  The E_pp class-diagonal blocks have their diagonal zeroed on device, making
  each diagonal entry contribute exactly exp(0)=1 per bandwidth; the exact
  correction (5*CAP - 5*exp(-1e-5)*cs) is applied as a host-computed offset.
  g2 is symmetric -> T2 = T1^T, so inter = sum_{s!=t} 2*(T1-T3)/(C^2-C).

Distribution (SPMD, one program, per-core data):
  - NO collective: an 8-core AllReduce has a ~95us floor in this
    environment, far more than recomputing the [12,12] global sums
    locally. Every core computes the FULL (rotated) E_ss (6 upper slabs),
    E_tt (block upper triangle) and E_pt in bf16 and derives the gamma
    sums via transpose-free two-stage small matmuls.
  - d2 is accumulated entirely in PSUM: Gram matmuls (bf16 in, f32 acc)
    plus a 1-partition rank-1 matmul that adds -0.5*rownorm (bf16) along
    the free dim; the Act engine then does E = sqrt(-2*psum + colnorm)
    in one pass per slab (scale=-2, bias=colnorm+eps f32). No Vector
    work in the E chain. eps=1.0 absorbs the bf16 row-norm rounding.
  - strict sqrt-then-exp Act ordering: exactly two ACT table loads.
    E_ss slabs are emitted first so S1 -> g2 -> sclT1 completes while
    E_tt/E_pt still stream; the T1 exp fires right after the table
    switch that follows the last sqrt.
  - exp work stays sharded: each core exponentiates only its own class
    pair's T1/T3 blocks (flat broadcast gathers through DRAM), its k3
    rows and its 5+3 (class,bandwidth) k2 units on the E_tt triangle
    (off-diag blocks weighted x2). k1 bandwidths 0-3 ride the spare
    partitions of the T3 pass; k1 bandwidth 4 is one tiny extra pass.
  - per-core weighted reduce with two host weight matrices (intra,
    inter) -> [intra, inter] partials, host sums the 8 partials.
"""

import math
import numpy as np

C = 12
KN = 5
MU = 2
N = 384
D = 256
CAP = 64
R = C * CAP            # 768 padded src rows
NCORES = 8
NCOL = 16              # ACC: T1, T3(+k1 k0-3), k1 k4, k3*5, k2q0*5, k2q1*3
DIAG5 = 5.0 * math.exp(-1e-5)
I2 = 2.0 / (C * C - C)
EPS = 1.0              # d2 positivity epsilon folded into the col norms

# mf: f32 full-height pack [128, MFW]
F_WMA = 0
F_WMB = 16
F_RSCOL6 = 32          # [128,6] per-slab src col norms (+eps)
F_RTCOL = 38           # [128,3] tgt col norms (+eps)
MFW = 41

# mb: bf16 full-height pack [128, MBW]
B_WR = 0               # 6 x [128,12]
B_PTB = 72             # 3 x [128,12]
B_DIAGM = 108
B_PCF = 172            # [128,6]
B_T1M = 178            # t1 mask [128,60]
B_T3M = 238            # t3 mask [128,70]
MBW = 308

# m12: f32 12-partition pack [12, M12W]
T_M = 0                # [12,128] partition-expand select
T_EYE = 128
T_K2SEL = 140
T_PW70 = 142
T_RDEN2 = 212
T_RDENIN = 224
T_OFFS = 225           # row 0: [1,2]
T_GINM = 227           # [12,36] gin mask over stage2 cols 24:60
M12W = 263

# mrow: bf16 1-partition pack [1, MRW]
R_NSROW = 0            # -0.5 * src row norms (768)
R_NTROW = 768          # -0.5 * tgt row norms (384)
R_PTR2A = 1152
R_PTR2B = 1536
R_PTR3A = 1920
R_PTR3B = 2304
MRW = 2688

TRI = (384, 256, 128)  # E_tt triangle slab widths
TRI_OFF = (0, 384, 640)
PTT_W = 640            # E_ptT band width (src rows 128:768)

_COMPILED = {}


# ----------------------------------------------------------------------------
# host-side prep
# ----------------------------------------------------------------------------

def _host_prep(src_x, tgt_x, src_y, tgt_y):
    import ml_dtypes
    bf16 = ml_dtypes.bfloat16
    src_x = np.ascontiguousarray(np.asarray(src_x, dtype=np.float32))
    tgt_x = np.ascontiguousarray(np.asarray(tgt_x, dtype=np.float32))
    src_y = np.asarray(src_y).astype(np.int64)
    pt = np.ascontiguousarray(np.asarray(tgt_y, dtype=np.float32))

    counts = np.bincount(src_y, minlength=C)
    if counts.max() > CAP:
        return None  # caller falls back to numpy path

    perm = np.argsort(src_y, kind="stable")
    sx_pad = np.zeros((R, D), np.float32)
    W = np.zeros((R, C), np.float32)
    # pad sentinels: huge random-sign vectors. Pad-pad dot products are then
    # tiny relative to the norms (no catastrophic cancellation in d2), every
    # pad-involved distance is >= ~3e5 and exp(-dist/bw) underflows to 0.
    rng = np.random.default_rng(987654321)
    sgn = (rng.integers(0, 2, size=(R, D)).astype(np.float32) * 2.0 - 1.0)
    off = 0
    padidx = 0
    padrow = np.zeros(R, bool)
    for c in range(C):
        idx = perm[off:off + counts[c]]
        sx_pad[c * CAP:c * CAP + counts[c]] = src_x[idx]
        W[c * CAP:c * CAP + counts[c], c] = 1.0
        padrow[c * CAP + counts[c]:(c + 1) * CAP] = True
        for p in range(CAP - counts[c]):
            sx_pad[c * CAP + counts[c] + p, :] = 2.0e4 * sgn[padidx]
            padidx += 1
        off += counts[c]
    # per-row d2 epsilon (col-norm side, f32): pad rows have ~1e11 norms
    # where the small eps vanishes; a 1e9 floor keeps sqrt safe and only
    # perturbs pad distances, whose exp underflows to 0 regardless
    eps_row = np.where(padrow, 1.0e9, EPS).astype(np.float32)

    # round features to bf16 host-side; norms are computed from the rounded
    # values in f32 so the d2 diagonal cancels to ~eps on device
    sx_bf = sx_pad.astype(bf16)
    tx_bf = tgt_x.astype(bf16)
    sx_rf = sx_bf.astype(np.float32)
    tx_rf = tx_bf.astype(np.float32)

    txT_pack = np.zeros((128, 768), bf16)
    for k in range(2):
        txT_pack[:, k * N:(k + 1) * N] = tx_bf.T[k * 128:(k + 1) * 128, :]
    rtrow = (tx_rf ** 2).sum(1)
    rtcol3 = np.zeros((128, 3), np.float32)
    for blk in range(3):
        rtcol3[:, blk] = rtrow[blk * 128:(blk + 1) * 128] + EPS
    ntrow_bf = (-0.5 * rtrow).astype(bf16)

    cs = counts.astype(np.float64)
    ct = pt.sum(0).astype(np.float64)
    pss = cs * cs
    ptt = ct * ct

    rden2 = (1.0 / (pss[:, None] + pss[None, :]
                    + 2.0 * cs[:, None] * cs[None, :])).astype(np.float32)
    rdenin = (1.0 / (pss + ptt + 2.0 * cs * ct)).astype(np.float32).reshape(C, 1)

    pw5 = np.array([-(float(MU) ** (k - KN // 2)) for k in range(KN)],
                   np.float32)
    pw70 = np.zeros((C, 70), np.float32)
    for k in range(KN):
        pw70[:, k * 12:(k + 1) * 12] = pw5[k]

    eye = np.eye(C, dtype=np.float32)
    ginmask = np.zeros((C, 36), np.float32)
    ginmask[:, 0:12] = 2.0 * eye     # U_tt diag (stt = U+U^T-BD)
    ginmask[:, 12:24] = -eye         # BD_tt diag
    ginmask[:, 24:36] = 2.0 * eye    # sst diag

    in_maps = []
    for r in range(NCORES):
        g = r % 6
        a, b = 2 * g, 2 * g + 1
        pp_active = r < 6
        roll = 2 * g * CAP

        sxr_bf = np.roll(sx_bf, -roll, axis=0)
        sxr_rf = np.roll(sx_rf, -roll, axis=0)
        sxT_pack = np.zeros((128, 1536), bf16)
        for k in range(2):
            sxT_pack[:, k * R:(k + 1) * R] = sxr_bf.T[k * 128:(k + 1) * 128, :]
        norms = (sxr_rf ** 2).sum(1)
        eps_r = np.roll(eps_row, -roll)
        rscol6 = (norms + eps_r).reshape(6, 128).T.astype(np.float32)
        nsrow_bf = (-0.5 * norms).astype(bf16)

        wr = np.roll(W, -roll, axis=0)

        # k2 split: q0 = class r with all 5 bandwidths; q1 = class 8+(r%4)
        # with bandwidths {0,1,2} on cores 0-3 and {3,4} on cores 4-7.
        c_q0 = r
        c_q1 = 8 + (r % 4)
        kq1 = [0, 1, 2] if r < 4 else [3, 4]
        k2sel = np.zeros((C, 2), np.float32)
        k2sel[c_q0, 0] = 1.0
        k2sel[c_q1, 1] = 1.0
        pw70r = pw70.copy()
        pw70r[:, 60:65] = pw5[None, :]
        for j in range(5):
            pw70r[:, 65 + j] = pw5[kq1[j]] if j < len(kq1) else pw5[0]

        ptcolf = np.zeros((128, 6), np.float32)
        for q, c in enumerate((c_q0, c_q1)):
            for blk in range(3):
                ptcolf[:, q * 3 + blk] = pt[blk * 128:(blk + 1) * 128, c]

        # partition-expand select: row cls -> partitions of that half
        M = np.zeros((C, 128), np.float32)
        M[a, 0:64] = 1.0
        M[b, 64:128] = 1.0

        # masks over sel128 [128,70] (= ibg[cls_p, :] per partition)
        t1mask = np.zeros((128, 60), np.float32)
        t3mask = np.zeros((128, 70), np.float32)
        for h in range(2):
            for k in range(KN):
                for t in range(12):
                    p = h * 64 + k * 12 + t
                    t1mask[p, k * 12 + t] = 1.0
                    t3mask[p, k * 12 + (2 * g + t) % 12] = 1.0
            for j in range(4):   # k1 bandwidths 0-3 ride the T3 pads
                t3mask[h * 64 + 60 + j, 60 + j] = 1.0

        wmA = np.zeros((128, NCOL), np.float32)   # intra weights
        wmB = np.zeros((128, NCOL), np.float32)   # inter weights
        if pp_active:
            for h, cls in ((0, a), (1, b)):
                for k in range(KN):
                    for t in range(12):
                        p = h * 64 + k * 12 + t
                        if t != cls:
                            wmB[p, 0] = I2 / pss[cls]
                        rt_ = (2 * g + t) % 12
                        if rt_ != cls:
                            wmB[p, 1] = -I2 / (cs[cls] * cs[rt_])
                for j in range(4):   # k1 k=0..3 in T3 pads (full-block sums)
                    wmA[h * 64 + 60 + j, 1] = 1.0 / (C * pss[cls])
                # k1 k=4: E_diag natural layout, per-row sums
                wmA[h * CAP:(h + 1) * CAP, 2] = 1.0 / (C * pss[cls])
                for k in range(KN):
                    wmA[h * CAP:(h + 1) * CAP, 3 + k] = \
                        -2.0 / (C * cs[cls] * ct[cls])
        wmA[:, 8:13] = 1.0 / (C * ptt[c_q0])
        for j in range(len(kq1)):
            wmA[:, 13 + j] = 1.0 / (C * ptt[c_q1])

        ssel = np.zeros((2 * NCOL, 2), np.float32)
        ssel[0:NCOL, 0] = 1.0       # wmA block -> intra
        ssel[NCOL:2 * NCOL, 1] = 1.0  # wmB block -> inter

        offs = np.zeros(2, np.float32)
        if r == 0:
            corr = 5.0 * CAP - DIAG5 * cs
            offs[0] = -(corr / pss / C).sum()
            offs[1] = -((C - 1) * corr * I2 / pss).sum()

        mf = np.zeros((128, MFW), np.float32)
        mf[:, F_WMA:F_WMA + NCOL] = wmA
        mf[:, F_WMB:F_WMB + NCOL] = wmB
        mf[:, F_RSCOL6:F_RSCOL6 + 6] = rscol6
        mf[:, F_RTCOL:F_RTCOL + 3] = rtcol3

        mb = np.zeros((128, MBW), np.float32)
        for m in range(6):
            mb[:, B_WR + m * 12:B_WR + (m + 1) * 12] = wr[m * 128:(m + 1) * 128]
        for m in range(3):
            mb[:, B_PTB + m * 12:B_PTB + (m + 1) * 12] = \
                pt[m * 128:(m + 1) * 128]
        mb[0:CAP, B_DIAGM:B_DIAGM + CAP] = 1.0 - np.eye(CAP)
        mb[CAP:128, B_DIAGM:B_DIAGM + CAP] = 1.0 - np.eye(CAP)
        mb[:, B_PCF:B_PCF + 6] = ptcolf
        mb[:, B_T1M:B_T1M + 60] = t1mask
        mb[:, B_T3M:B_T3M + 70] = t3mask

        m12 = np.zeros((12, M12W), np.float32)
        m12[:, T_M:T_M + 128] = M
        m12[:, T_EYE:T_EYE + 12] = eye
        m12[:, T_K2SEL:T_K2SEL + 2] = k2sel
        m12[:, T_PW70:T_PW70 + 70] = pw70r
        m12[:, T_RDEN2:T_RDEN2 + 12] = rden2
        m12[:, T_RDENIN:T_RDENIN + 1] = rdenin
        m12[0, T_OFFS:T_OFFS + 2] = offs
        m12[:, T_GINM:T_GINM + 36] = ginmask

        mrow = np.zeros((1, MRW), np.float32)
        mrow[0, R_NSROW:R_NSROW + R] = nsrow_bf.astype(np.float32)
        mrow[0, R_NTROW:R_NTROW + N] = ntrow_bf.astype(np.float32)
        mrow[0, R_PTR2A:R_PTR2A + N] = pt[:, c_q0]
        mrow[0, R_PTR2B:R_PTR2B + N] = pt[:, c_q1]
        mrow[0, R_PTR3A:R_PTR3A + N] = pt[:, a]
        mrow[0, R_PTR3B:R_PTR3B + N] = pt[:, b]

        in_maps.append({
            "sxT": sxT_pack,
            "txT": txT_pack,
            "mf": np.ascontiguousarray(mf),
            "mb": np.ascontiguousarray(mb.astype(bf16)),
            "m12": np.ascontiguousarray(m12),
            "mrow": np.ascontiguousarray(mrow.astype(bf16)),
            "msel": np.ascontiguousarray(ssel),
        })
    return in_maps


def _numpy_fallback(src_x, tgt_x, src_y, tgt_y):
    f = np.float32
    src_x = np.asarray(src_x, f)
    tgt_x = np.asarray(tgt_x, f)
    src_y = np.asarray(src_y).astype(np.int64)
    pt = np.asarray(tgt_y, f)
    ps = np.eye(C, dtype=f)[src_y]

    def cdist(a, bb):
        d2 = (a * a).sum(1)[:, None] + (bb * bb).sum(1)[None, :] - 2.0 * (a @ bb.T)
        return np.sqrt(np.maximum(d2, 0.0))

    def kern(dist, g):
        acc = 0.0
        for i in range(KN):
            bw = np.maximum(np.asarray(g) * (MU ** (i - KN // 2)), 1e-5)
            acc = acc + np.exp(-np.clip(dist / bw, 1e-5, 1e5))
        return acc

    E_ss = cdist(src_x, src_x); E_tt = cdist(tgt_x, tgt_x); E_st = cdist(src_x, tgt_x)
    sss = np.einsum('ic,ij,jc->c', ps, E_ss, ps)
    stt = np.einsum('ic,ij,jc->c', pt, E_tt, pt)
    sst = np.einsum('is,ij,jt->st', ps, E_st, pt)
    cs = ps.sum(0); ct = pt.sum(0)
    pss = cs * cs; ptt = ct * ct; pstd = cs * ct
    g_in = (sss + stt + 2 * np.diagonal(sst)) / (pss + ptt + 2 * pstd)
    Pss = ps.T[:, :, None] * ps.T[:, None, :]
    Ptt = pt.T[:, :, None] * pt.T[:, None, :]
    Pst = ps.T[:, :, None] * pt.T[:, None, :]
    k1 = (kern(E_ss[None] * Pss, g_in[:, None, None]) * Pss).sum((-2, -1)) / pss
    k2 = (kern(E_tt[None] * Ptt, g_in[:, None, None]) * Ptt).sum((-2, -1)) / ptt
    k3 = (kern(E_st[None] * Pst, g_in[:, None, None]) * Pst).sum((-2, -1)) / pstd
    intra = (k1 + k2 - 2 * k3).sum() / C
    sst_s = np.einsum('is,ij,jt->st', ps, E_ss, ps)
    g2 = (sss[:, None] + sss[None, :] + 2 * sst_s) / (
        pss[:, None] + pss[None, :] + 2 * cs[:, None] * cs[None, :])
    T1 = np.zeros((C, C), f); T3 = np.zeros((C, C), f)
    for s in range(C):
        ms = ps[:, s].astype(bool)
        for t in range(C):
            mt = ps[:, t].astype(bool)
            T1[s, t] = kern(E_ss[np.ix_(ms, ms)], g2[s, t]).sum() / pss[s]
            T3[s, t] = kern(E_ss[np.ix_(ms, mt)], g2[s, t]).sum() / (cs[s] * cs[t])
    inter = ((2 * T1 - 2 * T3) * (1 - np.eye(C))).sum() / (C * C - C)
    return np.array([intra, inter], np.float32)


# ----------------------------------------------------------------------------
# device program
# ----------------------------------------------------------------------------

def _build_program():
    import concourse.bass as bass
    import concourse.tile as tile
    from concourse import bacc, mybir

    f32 = mybir.dt.float32
    bf = mybir.dt.bfloat16
    AF = mybir.ActivationFunctionType
    OP = mybir.AluOpType

    nc = bacc.Bacc("TRN2", target_bir_lowering=False, debug=False,
                   num_devices=NCORES)

    i_sxT = nc.dram_tensor("sxT", [128, 2 * R], bf, kind="ExternalInput").ap()
    i_txT = nc.dram_tensor("txT", [128, 2 * N], bf, kind="ExternalInput").ap()
    i_mf = nc.dram_tensor("mf", [128, MFW], f32, kind="ExternalInput").ap()
    i_mb = nc.dram_tensor("mb", [128, MBW], bf, kind="ExternalInput").ap()
    i_m12 = nc.dram_tensor("m12", [12, M12W], f32, kind="ExternalInput").ap()
    i_mrow = nc.dram_tensor("mrow", [1, MRW], bf, kind="ExternalInput").ap()
    i_msel = nc.dram_tensor("msel", [2 * NCOL, 2], f32,
                            kind="ExternalInput").ap()

    o_out = nc.dram_tensor("out", [1, 2], f32, kind="ExternalOutput").ap()

    with tile.TileContext(nc) as tc:
        with (
            tc.tile_pool(name="io", bufs=1) as io,
            tc.tile_pool(name="big", bufs=1) as big,
            tc.tile_pool(name="scr", bufs=2) as scr,
            tc.tile_pool(name="sm", bufs=1) as sm,
            tc.tile_pool(name="pG", bufs=2, space="PSUM") as pG,
            tc.tile_pool(name="p1", bufs=1, space="PSUM") as p1,
            tc.tile_pool(name="pT", bufs=2, space="PSUM") as pT,
            tc.tile_pool(name="dram", bufs=1, space="DRAM") as dpool,
        ):
            # ---------------- input loads, spread across queues -----------
            sxT = io.tile([128, 2 * R], bf, tag="sxT", name="sxT")
            nc.sync.dma_start(out=sxT[:, 0:R], in_=i_sxT[:, 0:R])
            nc.gpsimd.dma_start(out=sxT[:, R:2 * R], in_=i_sxT[:, R:2 * R])
            mf = io.tile([128, MFW], f32, tag="mf", name="mf")
            nc.sync.dma_start(out=mf[:], in_=i_mf[:])
            txT = io.tile([128, 2 * N], bf, tag="txT", name="txT")
            nc.scalar.dma_start(out=txT[:], in_=i_txT[:])
            mrow = io.tile([1, MRW], bf, tag="mrow", name="mrow")
            nc.scalar.dma_start(out=mrow[:], in_=i_mrow[:])
            m12 = io.tile([12, M12W], f32, tag="m12", name="m12")
            nc.scalar.dma_start(out=m12[:], in_=i_m12[:])
            mb = io.tile([128, MBW], bf, tag="mb", name="mb")
            nc.gpsimd.dma_start(out=mb[:], in_=i_mb[:])
            msel = io.tile([2 * NCOL, 2], f32, tag="msel", name="msel")
            nc.sync.dma_start(out=msel[:], in_=i_msel[:])

            wmAB = mf[:, F_WMA:F_WMA + 2 * NCOL]
            rscol6 = mf[:, F_RSCOL6:F_RSCOL6 + 6]
            rtcol = mf[:, F_RTCOL:F_RTCOL + 3]

            wrb = [mb[:, B_WR + m * 12:B_WR + (m + 1) * 12] for m in range(6)]
            ptb = [mb[:, B_PTB + m * 12:B_PTB + (m + 1) * 12] for m in range(3)]
            diagm = mb[:, B_DIAGM:B_DIAGM + CAP]
            pcf = mb[:, B_PCF:B_PCF + 6]
            t1mask = mb[:, B_T1M:B_T1M + 60]
            t3mask = mb[:, B_T3M:B_T3M + 70]

            Msel = m12[0:12, T_M:T_M + 128]
            eye12 = m12[0:12, T_EYE:T_EYE + 12]
            k2sel = m12[0:12, T_K2SEL:T_K2SEL + 2]
            pw60 = m12[0:12, T_PW70:T_PW70 + 60]
            pw10 = m12[0:12, T_PW70 + 60:T_PW70 + 70]
            rden2 = m12[0:12, T_RDEN2:T_RDEN2 + 12]
            rdenin = m12[0:12, T_RDENIN:T_RDENIN + 1]
            offs = m12[0:1, T_OFFS:T_OFFS + 2]
            ginmask = m12[0:12, T_GINM:T_GINM + 36]

            nsrow = mrow[0:1, R_NSROW:R_NSROW + R]
            ntrow = mrow[0:1, R_NTROW:R_NTROW + N]
            ptr2 = [mrow[0:1, R_PTR2A:R_PTR2A + N],
                    mrow[0:1, R_PTR2B:R_PTR2B + N]]
            ptr3 = [mrow[0:1, R_PTR3A:R_PTR3A + N],
                    mrow[0:1, R_PTR3B:R_PTR3B + N]]

            ones = sm.tile([128, 1], f32, tag="ones", name="ones")
            nc.vector.memset(ones[:], 1.0)
            onesr = sm.tile([1, 128], f32, tag="onesr", name="onesr")
            nc.vector.memset(onesr[:], 1.0)
            onesb = sm.tile([1, 128], bf, tag="onesb", name="onesb")
            nc.vector.memset(onesb[:], 1.0)
            acc = big.tile([128, NCOL], f32, tag="acc", name="acc")
            nc.vector.memset(acc[:], 0.0)

            # early gpsimd broadcasts (only need mrow)
            ptrow2b = [big.tile([128, N], bf, tag=f"ptrow2b{q}",
                                name=f"ptrow2b{q}") for q in range(2)]
            nc.gpsimd.partition_broadcast(ptrow2b[0][:], ptr2[0])
            nc.gpsimd.partition_broadcast(ptrow2b[1][:], ptr2[1])
            ptw3 = big.tile([128, N], bf, tag="ptw3", name="ptw3")
            ptw3t = big.tile([128, N], bf, tag="ptw3t", name="ptw3t")
            nc.gpsimd.partition_broadcast(ptw3[:], ptr3[0])
            nc.gpsimd.partition_broadcast(ptw3t[:], ptr3[1])
            nc.vector.tensor_copy(ptw3[CAP:128, :], ptw3t[CAP:128, :])

            sxTk = [sxT[:, 0:R], sxT[:, R:2 * R]]
            txTk = [txT[:, 0:N], txT[:, N:2 * N]]

            # ---------------- E emission: Gram + rank1 in PSUM, sqrt ------
            def emit_E(dst_ap, lhsT_k, lhs_lo, rhs_k, n_cols, rhs_lo,
                       rank1_row, bias_ap):
                # d2 accumulates in PSUM: sum_k lhsT^T rhs  - 0.5*rownorm;
                # Act: E = sqrt(-2*psum + colnorm_bias)
                gp = pG.tile([128, 1024], f32, tag="G", name="gp")
                done = 0
                while done < n_cols:
                    nchunk = min(512, n_cols - done)
                    sl = gp[:, done:done + nchunk]
                    for k in range(2):
                        nc.tensor.matmul(
                            sl,
                            lhsT_k[k][:, lhs_lo:lhs_lo + 128],
                            rhs_k[k][:, rhs_lo + done:rhs_lo + done + nchunk],
                            start=(k == 0), stop=False)
                    nc.tensor.matmul(
                        sl, onesb[:],
                        rank1_row[0:1, done:done + nchunk],
                        start=False, stop=True)
                    done += nchunk
                nc.scalar.activation(dst_ap, gp[:, 0:n_cols], AF.Sqrt,
                                     bias=bias_ap, scale=-2.0)

            # E_ss upper slabs first (feed S1 -> g2 -> sclT1)
            E_ss = [big.tile([128, R - 128 * s], bf, tag=f"E_ss{s}",
                             name=f"E_ss{s}") for s in range(6)]
            # one PSUM bank holds all small-sum regions (p1 tile [128,372]):
            #   stage-1: 0:72 ups, 72:144 bd, 144:180 U_tt, 180:216 BD_tt,
            #            216:312 sst
            #   stage-2 [12 parts]: 312:324 U_ss, 324:336 BD_ss, 336:348 U_tt,
            #            348:360 BD_tt, 360:372 sst
            s1p = p1.tile([128, 372], f32, tag="s1p", name="s1p")
            for s in range(6):
                r1 = bass.AP(tensor=nsrow.tensor, offset=nsrow.offset + 128 * s,
                             ap=[list(nsrow.ap[0]), [1, R - 128 * s]])
                emit_E(E_ss[s][:], sxTk, s * 128, sxTk, R - 128 * s, 128 * s,
                       r1, rscol6[:, s:s + 1])
                # stage-1 ups: pp[s] = sum_{slab<=s} E_ss[slab] chunk @ wr
                for slab in range(s + 1):
                    nc.tensor.matmul(
                        s1p[:, 12 * s:12 * s + 12],
                        E_ss[slab][:, (s - slab) * 128:(s - slab + 1) * 128],
                        wrb[slab], start=(slab == 0), stop=(slab == s))
                # stage-1 bd: block-diagonal term
                nc.tensor.matmul(s1p[:, 72 + 12 * s:72 + 12 * s + 12],
                                 E_ss[s][:, 0:128], wrb[s],
                                 start=True, stop=True)

            E_own = E_ss[0]

            # E_own diag-zero + DRAM bounce for the T1/T3 gathers
            E_diag = big.tile([128, CAP], bf, tag="E_diag", name="E_diag")
            nc.vector.tensor_tensor(E_diag[0:CAP, :], E_own[0:CAP, 0:CAP],
                                    diagm[0:CAP, :], OP.mult)
            nc.vector.tensor_tensor(E_diag[CAP:128, :],
                                    E_own[CAP:128, CAP:128],
                                    diagm[CAP:128, :], OP.mult)

            d_ed = dpool.tile([128, CAP], bf, tag="d_ed", name="d_ed")
            nc.sync.dma_start(out=d_ed[:], in_=E_diag[:])
            # E_own -> DRAM in class-block layout [t][row][col] so each
            # (half, t) block is one contiguous segment
            d_eob = dpool.tile([C, 128 * CAP], bf, tag="d_eob", name="d_eob")
            dst_eob = bass.AP(tensor=d_eob.tensor, offset=0,
                              ap=[[CAP, 128], [128 * CAP, 12], [1, CAP]])
            nc.gpsimd.dma_start(out=dst_eob, in_=E_own[:])

            t1src = big.tile([128, CAP * CAP], bf, tag="t1src", name="t1src")
            for h in range(2):
                ap_in = bass.AP(tensor=d_ed.tensor, offset=h * CAP * CAP,
                                ap=[[0, 64], [1, CAP * CAP]])
                nc.sync.dma_start(out=t1src[h * 64:(h + 1) * 64, :], in_=ap_in)

            # T3 flat blocks: partition h*64 + k*12 + t <- block (h, t);
            # pads read the zeroed flat block (they carry k1 k=0..3)
            t3src = big.tile([128, CAP * CAP], bf, tag="t3src", name="t3src")
            for h in range(2):
                ap_in = bass.AP(tensor=d_eob.tensor, offset=h * CAP * CAP,
                                ap=[[0, 5], [128 * CAP, 12], [1, CAP * CAP]])
                nc.gpsimd.dma_start(out=t3src[h * 64:h * 64 + 60, :], in_=ap_in)
                ap_pad = bass.AP(tensor=d_ed.tensor, offset=h * CAP * CAP,
                                 ap=[[0, 4], [1, CAP * CAP]])
                nc.gpsimd.dma_start(out=t3src[h * 64 + 60:h * 64 + 64, :],
                                    in_=ap_pad)

            # ---------------- E_tt block upper triangle -------------------
            # layout [128, 768]: cols 0:384 band0 rows x tgt blocks 0..2,
            # 384:640 band1 x blocks 1..2, 640:768 band2 x block 2
            E_tri = big.tile([128, 768], bf, tag="E_tri", name="E_tri")
            for bnd in range(3):
                r1 = bass.AP(tensor=ntrow.tensor,
                             offset=ntrow.offset + 128 * bnd,
                             ap=[list(ntrow.ap[0]), [1, N - 128 * bnd]])
                emit_E(E_tri[:, TRI_OFF[bnd]:TRI_OFF[bnd] + TRI[bnd]],
                       txTk, bnd * 128, txTk, N - 128 * bnd, 128 * bnd,
                       r1, rtcol[:, bnd:bnd + 1])
            # stage-1 U_tt / BD_tt
            for sub in range(3):
                for bnd in range(sub + 1):
                    nc.tensor.matmul(
                        s1p[:, 144 + 12 * sub:144 + 12 * sub + 12],
                        E_tri[:, TRI_OFF[bnd] + (sub - bnd) * 128:
                              TRI_OFF[bnd] + (sub - bnd + 1) * 128],
                        ptb[bnd], start=(bnd == 0), stop=(bnd == sub))
                nc.tensor.matmul(s1p[:, 180 + 12 * sub:180 + 12 * sub + 12],
                                 E_tri[:, TRI_OFF[sub]:TRI_OFF[sub] + 128],
                                 ptb[sub], start=True, stop=True)

            # k2 P / Pw / D tiles on the triangle layout
            k2P = []
            k2Pw = []
            k2D = []
            for q in range(2):
                P = big.tile([128, 768], bf, tag=f"k2P{q}", name=f"k2P{q}")
                Pw = big.tile([128, 768], bf, tag=f"k2Pw{q}", name=f"k2Pw{q}")
                for bnd in range(3):
                    w = TRI[bnd]
                    colap = bass.AP(tensor=pcf.tensor,
                                    offset=pcf.offset + q * 3 + bnd,
                                    ap=[list(pcf.ap[0]), [0, w]])
                    rowap = bass.AP(
                        tensor=ptrow2b[q].tensor,
                        offset=ptrow2b[q].offset + 128 * bnd,
                        ap=[list(ptrow2b[q].ap[0]), [1, w]])
                    nc.vector.tensor_tensor(
                        P[:, TRI_OFF[bnd]:TRI_OFF[bnd] + w],
                        colap, rowap, OP.mult)
                    # weighted copy: diag block x1, off-diag blocks x2
                    nc.vector.tensor_copy(
                        Pw[:, TRI_OFF[bnd]:TRI_OFF[bnd] + 128],
                        P[:, TRI_OFF[bnd]:TRI_OFF[bnd] + 128])
                    if w > 128:
                        nc.vector.tensor_scalar(
                            Pw[:, TRI_OFF[bnd] + 128:TRI_OFF[bnd] + w],
                            P[:, TRI_OFF[bnd] + 128:TRI_OFF[bnd] + w],
                            2.0, None, OP.mult)
                Dt = big.tile([128, 768], bf, tag=f"k2D{q}", name=f"k2D{q}")
                nc.vector.tensor_tensor(Dt[:], E_tri[:], P[:], OP.mult)
                k2P.append(P)
                k2Pw.append(Pw)
                k2D.append(Dt)

            # ---------------- E_pt: slab0 src-major, rest tgt-major -------
            E_pt0 = big.tile([128, N], bf, tag="E_pt0", name="E_pt0")
            r1t = bass.AP(tensor=ntrow.tensor, offset=ntrow.offset,
                          ap=[list(ntrow.ap[0]), [1, N]])
            emit_E(E_pt0[:], sxTk, 0, txTk, N, 0, r1t, rscol6[:, 0:1])

            k3D = big.tile([128, N], bf, tag="k3D", name="k3D")
            nc.vector.tensor_tensor(k3D[:], E_pt0[:], ptw3[:], OP.mult)

            # sst stage-1 from slab0 (per tgt block, wr-weighted)
            for bblk in range(3):
                nc.tensor.matmul(s1p[:, 216 + 12 * bblk:216 + 12 * bblk + 12],
                                 E_pt0[:, bblk * 128:(bblk + 1) * 128],
                                 wrb[0], start=True, stop=True)

            # E_ptT: tgt-major bands covering src rows 128:768
            E_ptT = big.tile([128, 3 * PTT_W], bf, tag="E_ptT", name="E_ptT")
            for bnd in range(3):
                r1 = bass.AP(tensor=nsrow.tensor, offset=nsrow.offset + 128,
                             ap=[list(nsrow.ap[0]), [1, PTT_W]])
                emit_E(E_ptT[:, bnd * PTT_W:(bnd + 1) * PTT_W],
                       txTk, bnd * 128, sxTk, PTT_W, 128,
                       r1, rtcol[:, bnd:bnd + 1])
            # sst stage-1 from T bands (per src block, pt-weighted)
            for s in range(5):
                for bnd in range(3):
                    nc.tensor.matmul(
                        s1p[:, 252 + 12 * s:252 + 12 * s + 12],
                        E_ptT[:, bnd * PTT_W + s * 128:
                              bnd * PTT_W + (s + 1) * 128],
                        ptb[bnd], start=(bnd == 0), stop=(bnd == 2))

            # ---------------- stage-2 sums into the same PSUM bank --------
            s2p = s1p[0:12, 312:372]
            cb1 = scr.tile([128, 144], bf, tag="cb1", name="cb1")
            nc.vector.tensor_copy(cb1[:], s1p[:, 0:144])
            for s in range(6):
                nc.tensor.matmul(s2p[:, 0:12], cb1[:, 12 * s:12 * s + 12],
                                 wrb[s], start=(s == 0), stop=(s == 5))
            for s in range(6):
                nc.tensor.matmul(s2p[:, 12:24], cb1[:, 72 + 12 * s:84 + 12 * s],
                                 wrb[s], start=(s == 0), stop=(s == 5))
            cb2 = scr.tile([128, 72], bf, tag="cb2", name="cb2")
            nc.vector.tensor_copy(cb2[:], s1p[:, 144:216])
            for sub in range(3):
                nc.tensor.matmul(s2p[:, 24:36], cb2[:, 12 * sub:12 * sub + 12],
                                 ptb[sub], start=(sub == 0), stop=(sub == 2))
            for sub in range(3):
                nc.tensor.matmul(s2p[:, 36:48],
                                 cb2[:, 36 + 12 * sub:48 + 12 * sub],
                                 ptb[sub], start=(sub == 0), stop=(sub == 2))
            cb3 = scr.tile([128, 96], bf, tag="cb3", name="cb3")
            nc.vector.tensor_copy(cb3[:], s1p[:, 216:312])
            for bblk in range(3):
                nc.tensor.matmul(s2p[:, 48:60],
                                 cb3[:, 12 * bblk:12 * bblk + 12],
                                 ptb[bblk], start=(bblk == 0), stop=False)
            for s in range(5):
                nc.tensor.matmul(s2p[:, 48:60], wrb[s + 1],
                                 cb3[:, 36 + 12 * s:48 + 12 * s],
                                 start=False, stop=(s == 4))

            # ---------------- gammas: S1 -> g2 -> ibg -> scales -----------
            usb = sm.tile([12, 12], f32, tag="usb", name="usb")
            nc.vector.tensor_copy(usb[:], s2p[:, 0:12])
            utp = pT.tile([12, 12], f32, tag="tiny", name="utp")
            nc.tensor.transpose(utp[:], usb[:], eye12)
            S1 = sm.tile([12, 12], f32, tag="S1", name="S1")
            nc.vector.scalar_tensor_tensor(
                out=S1[:], in0=utp[:], scalar=1.0, in1=usb[:],
                op0=OP.mult, op1=OP.add)
            nc.vector.tensor_tensor(S1[:], S1[:], s2p[:, 12:24], OP.subtract)

            dscr = scr.tile([12, 36], f32, tag="dscr", name="dscr")
            ssscol = sm.tile([12, 1], f32, tag="ssscol", name="ssscol")
            nc.vector.tensor_tensor(dscr[:, 0:12], S1[:], eye12, OP.mult)
            nc.vector.reduce_sum(out=ssscol[:], in_=dscr[:, 0:12],
                                 axis=mybir.AxisListType.X)

            ssst = pT.tile([1, 12], f32, tag="tiny", name="ssst")
            nc.tensor.transpose(ssst[:], ssscol[:], eye12)
            ssstsb = sm.tile([1, 12], f32, tag="ssstsb", name="ssstsb")
            nc.vector.tensor_copy(ssstsb[:], ssst[:])
            ps12 = pT.tile([12, 12], f32, tag="tiny", name="ps12")
            nc.tensor.matmul(ps12[:], onesr[0:1, 0:12], ssstsb[:],
                             start=True, stop=True)
            g2 = sm.tile([12, 12], f32, tag="g2", name="g2")
            nc.vector.scalar_tensor_tensor(
                out=g2[:], in0=S1[:], scalar=2.0, in1=ps12[:],
                op0=OP.mult, op1=OP.add)
            nc.vector.scalar_tensor_tensor(
                out=g2[:], in0=g2[:], scalar=ssscol[:], in1=rden2,
                op0=OP.add, op1=OP.mult)

            ibg = sm.tile([12, 70], f32, tag="ibg", name="ibg")
            t60 = scr.tile([12, 70], f32, tag="t60", name="t60")
            g2ap = g2[:]
            g2exp = bass.AP(tensor=g2ap.tensor, offset=g2ap.offset,
                            ap=[list(g2ap.ap[0]), [0, 5], [1, 12]])
            nc.vector.tensor_tensor(t60[:, 0:60], g2exp, pw60, OP.mult)
            nc.vector.tensor_scalar(t60[:, 0:60], t60[:, 0:60], -1e-5, None,
                                    OP.min)
            nc.vector.reciprocal(ibg[:, 0:60], t60[:, 0:60])

            # sel128[p, :] = ibg[cls_p, :]; T1 scales need only g2 cols
            sel128 = pT.tile([128, 70], f32, tag="sel", name="sel128",
                             bufs=1)
            nc.tensor.matmul(sel128[:, 0:60], Msel, ibg[:, 0:60],
                             start=True, stop=True)
            selscr = scr.tile([128, 70], f32, tag="selscr", name="selscr")
            sclT1 = sm.tile([128, 1], f32, tag="sclT1", name="sclT1")
            nc.vector.tensor_tensor(selscr[:, 0:60], sel128[:, 0:60], t1mask,
                                    OP.mult)
            nc.vector.reduce_sum(out=sclT1[:], in_=selscr[:, 0:60],
                                 axis=mybir.AxisListType.X)

            # gin path (needs stt/sst stage-2): numerator via one masked
            # reduce over s2p cols 24:60, then  gin=(num+sss)*rdenin
            ginp = sm.tile([12, 1], f32, tag="ginp", name="ginp")
            nc.vector.tensor_tensor(dscr[:, 0:36], s2p[:, 24:60], ginmask,
                                    OP.mult)
            nc.vector.reduce_sum(out=ginp[:], in_=dscr[:, 0:36],
                                 axis=mybir.AxisListType.X)
            gin = sm.tile([12, 1], f32, tag="gin", name="gin")
            nc.vector.scalar_tensor_tensor(
                out=gin[:], in0=ginp[:], scalar=ssscol[:], in1=rdenin,
                op0=OP.add, op1=OP.mult)

            ginap = gin[:]
            ginexp = bass.AP(tensor=ginap.tensor, offset=ginap.offset,
                             ap=[list(ginap.ap[0]), [0, 10]])
            nc.vector.tensor_tensor(t60[:, 60:70], ginexp, pw10, OP.mult)
            nc.vector.tensor_scalar(t60[:, 60:70], t60[:, 60:70], -1e-5, None,
                                    OP.min)
            nc.vector.reciprocal(ibg[:, 60:70], t60[:, 60:70])
            nc.tensor.matmul(sel128[:, 60:70], Msel, ibg[:, 60:70],
                             start=True, stop=True)

            sclT3 = sm.tile([128, 1], f32, tag="sclT3", name="sclT3")
            nc.vector.tensor_tensor(selscr[:], sel128[:], t3mask, OP.mult)
            nc.vector.reduce_sum(out=sclT3[:], in_=selscr[:],
                                 axis=mybir.AxisListType.X)
            negk1 = sm.tile([128, 5], f32, tag="negk1", name="negk1")
            nc.vector.tensor_copy(negk1[:], sel128[:, 60:65])

            negb = []
            for q in range(2):
                k2sc = pT.tile([1, 5], f32, tag="tiny", name="k2sc")
                nc.tensor.matmul(k2sc[:], k2sel[:, q:q + 1],
                                 ibg[:, 60 + 5 * q:65 + 5 * q],
                                 start=True, stop=True)
                k2scsb = sm.tile([1, 5], f32, tag=f"k2scsb{q}",
                                 name=f"k2scsb{q}")
                nc.vector.tensor_copy(k2scsb[:], k2sc[:])
                pnb = pT.tile([128, 5], f32, tag="tiny", name="pnb")
                nc.tensor.matmul(pnb[:], onesr, k2scsb[:],
                                 start=True, stop=True)
                nb = sm.tile([128, 5], f32, tag=f"negb{q}", name=f"negb{q}")
                nc.vector.tensor_copy(nb[:], pnb[:])
                negb.append(nb)

            # ---------------- exp passes (one table switch before T1) -----
            e1 = scr.tile([128, CAP * CAP], bf, tag="e1", name="e1")
            nc.scalar.activation(e1[:], t1src[:], AF.Exp, scale=sclT1[:],
                                 accum_out=acc[:, 0:1])
            e3 = scr.tile([128, CAP * CAP], bf, tag="e3", name="e3")
            nc.scalar.activation(e3[:], t3src[:], AF.Exp, scale=sclT3[:],
                                 accum_out=acc[:, 1:2])
            ek1 = scr.tile([128, CAP], bf, tag="ek1", name="ek1")
            nc.scalar.activation(ek1[:], E_diag[:], AF.Exp,
                                 scale=negk1[:, 4:5], accum_out=acc[:, 2:3])

            for k in range(KN):
                ek = scr.tile([128, N], bf, tag="k3e", name="ek3")
                nc.scalar.activation(ek[:], k3D[:], AF.Exp,
                                     scale=negk1[:, k:k + 1])
                sk = scr.tile([128, N], bf, tag="k3s", name="sk3")
                nc.vector.scalar_tensor_tensor(
                    out=sk[:], in0=ek[:], scalar=1.0, in1=ptw3[:],
                    op0=OP.mult, op1=OP.mult,
                    accum_out=acc[:, 3 + k:4 + k])

            for q in range(2):
                npass = 5 if q == 0 else 3
                for j in range(npass):
                    ek = scr.tile([128, 768], bf, tag="k2e", name="ek2")
                    nc.scalar.activation(ek[:], k2D[q][:], AF.Exp,
                                         scale=negb[q][:, j:j + 1])
                    sk = scr.tile([128, 768], bf, tag="k2s", name="sk2")
                    col = 8 + 5 * q + j
                    nc.vector.scalar_tensor_tensor(
                        out=sk[:], in0=ek[:], scalar=1.0, in1=k2Pw[q][:],
                        op0=OP.mult, op1=OP.mult,
                        accum_out=acc[:, col:col + 1])

            # ---------------- final weighted reduce -----------------------
            v = big.tile([128, 2 * NCOL], f32, tag="v", name="v")
            accrep = bass.AP(tensor=acc.tensor, offset=acc.offset,
                             ap=[list(acc.ap[0]), [0, 2], [1, NCOL]])
            nc.vector.tensor_tensor(v[:], accrep, wmAB, OP.mult)
            m1 = pT.tile([2 * NCOL, 1], f32, tag="tiny", name="m1")
            nc.tensor.matmul(m1[:], v[:], ones, start=True, stop=True)
            m1sb = sm.tile([2 * NCOL, 1], f32, tag="m1sb", name="m1sb")
            nc.vector.tensor_copy(m1sb[:], m1[:])
            m2 = pT.tile([1, 2], f32, tag="tiny", name="m2")
            nc.tensor.matmul(m2[:], m1sb[:], msel[:], start=True, stop=True)
            res = sm.tile([1, 2], f32, tag="res", name="res")
            nc.vector.tensor_tensor(res[:], m2[:], offs, OP.add)
            nc.sync.dma_start(out=o_out[:], in_=res[:])

    nc.compile()
    return nc


def get_program():
    if "nc" not in _COMPILED:
        _COMPILED["nc"] = _build_program()
    return _COMPILED["nc"]


# ----------------------------------------------------------------------------
# entry point
# ----------------------------------------------------------------------------

def _run(in_maps, trace=False):
    from concourse.bass_utils import run_bass_kernel_spmd
    nc = get_program()
    return run_bass_kernel_spmd(nc, in_maps, list(range(NCORES)), trace=trace)


def kernel(src_x, tgt_x, src_y, tgt_y):
    in_maps = _host_prep(src_x, tgt_x, src_y, tgt_y)
    if in_maps is None:
        return _numpy_fallback(src_x, tgt_x, src_y, tgt_y)
    br = _run(in_maps)
    total = np.zeros(2, np.float64)
    for res in br.results:
        total += res["out"].reshape(2).astype(np.float64)
    return total.astype(np.float32)
